# revision 19
# baseline (speedup 1.0000x reference)
"""ArbSR (moe_routing) Trainium2 kernel, 8-core SPMD.

Structure exploited: with scale=4, the scale-embedding MLP input is periodic
with period 4 in both HR axes, so routing r, offsets off, and the expert-mix
matrices take only 16 distinct values (one per (y%4, x%4) class).  The
offset grid_sample then becomes, per class, a 2x2-tap bilinear filter of the
encoder feature map f at a constant integer shift, and the whole
  fea0 -> expert mixing -> (+fea0) -> 3x3 tail conv
chain collapses to
  pred[:, 4*yl+b, 4*xl+a] = tail_b + sum_delta E[(b,a)][delta] @ f[:, yl+dy, xl+dx]
with host-precomputed [3,64] matrices E (a 3x3 delta neighborhood in
practice).  Tail-conv zero padding at the image border is handled with
per-edge correction streams whose matrices are zeroed on cores that don't
own the edge.

Per core (64 HR rows): encoder conv as one K=28 matmul from a host-built
im2col; the pred computation as ~6 K=128-packed float32r matmul streams (f
stacked with a row-shifted copy); PE transpose into a pixel-major layout;
and an indirect-DMA gather for the nearest-neighbour queries, which the host
routes to cores by row ownership.
"""

import numpy as np


def _ensure_path():
    import sys
    for p in ('/opt/trn_rl_repo',):
        if p not in sys.path:
            sys.path.append(p)


H = W = 128
S = 4
HH = WH = H * S          # 512
C = 64
NCORES = 8
YLC = H // NCORES        # 16 LR rows per core
HRPC = HH // NCORES      # 64 HR rows per core
NPIX = HRPC * WH         # 32768 HR pixels per core
NQ_COLS = 264
NQP = 128 * NQ_COLS      # 33792 padded queries per core
NCLS = 16                # (b, a) classes
MROWS = NCLS * 3         # 48 stacked pred rows


def _sigmoid(x):
    return 1.0 / (1.0 + np.exp(-x))


def _class_constants(d):
    w1 = np.asarray(d['body_w1'], np.float64)
    b1 = np.asarray(d['body_b1'], np.float64)
    w2 = np.asarray(d['body_w2'], np.float64)
    b2 = np.asarray(d['body_b2'], np.float64)
    rw = np.asarray(d['routing_w'], np.float64)
    rb = np.asarray(d['routing_b'], np.float64)
    ow = np.asarray(d['offset_w'], np.float64)
    ob = np.asarray(d['offset_b'], np.float64)
    wc = np.asarray(d['weight_compress'], np.float64)
    we = np.asarray(d['weight_expand'], np.float64)

    fs = float(S)
    coor = np.array([(i + 0.5) / fs - np.floor((i + 0.5) / fs + 0.001) - 0.5
                     for i in range(S)])
    cls = {}
    for b in range(S):
        for a in range(S):
            inp4 = np.array([1.0 / fs, 1.0 / fs, coor[b], coor[a]])
            emb = np.maximum(w1 @ inp4 + b1, 0.0)
            emb = np.maximum(w2 @ emb + b2, 0.0)
            off = ow @ emb + ob
            r = _sigmoid(rw @ emb + rb)
            A = np.einsum('e,eck->ck', r, we) @ np.einsum('e,ekc->kc', r, wc)
            B = A + np.eye(C)
            cx = (a + 0.5) / fs - 0.5 + off[0]
            cy = (b + 0.5) / fs - 0.5 + off[1]
            ix, iy = int(np.floor(cx)), int(np.floor(cy))
            fx, fy = cx - ix, cy - iy
            wbl = {(0, 0): (1 - fy) * (1 - fx), (0, 1): (1 - fy) * fx,
                   (1, 0): fy * (1 - fx), (1, 1): fy * fx}
            cls[(b, a)] = dict(B=B, ix=ix, iy=iy, wbl=wbl)
    return cls


def _build_E(tail_w, cls, only_ty=None, only_tx=None):
    """E[(b,a)][(dy,dx)] = [3, C] so that pred contribution is E @ f(shift)."""
    Es = {}
    for b in range(S):
        for a in range(S):
            acc = {}
            for ty in range(3):
                if only_ty is not None and ty not in only_ty:
                    continue
                for tx in range(3):
                    if only_tx is not None and tx not in only_tx:
                        continue
                    bp = (b + ty - 1) % S
                    oy = (b + ty - 1 - bp) // S
                    ap_ = (a + tx - 1) % S
                    ox = (a + tx - 1 - ap_) // S
                    c2 = cls[(bp, ap_)]
                    TB = tail_w[:, :, ty, tx] @ c2['B']
                    for (uy, ux), wgt in c2['wbl'].items():
                        if wgt == 0.0:
                            continue
                        key = (oy + c2['iy'] + uy, ox + c2['ix'] + ux)
                        acc[key] = acc.get(key, np.zeros((3, C))) + TB * wgt
            Es[(b, a)] = acc
    return Es


def _stack_E(Es, deltas, classes=None, sign=1.0):
    """Per-delta [MROWS, C] matrices, rows ordered (4b+a)*3 + c."""
    G = {dl: np.zeros((MROWS, C)) for dl in deltas}
    for (b, a), acc in Es.items():
        if classes is not None and (b, a) not in classes:
            continue
        m0 = (4 * b + a) * 3
        for dl, M in acc.items():
            G[dl][m0:m0 + 3, :] += sign * M
    return G


def _pair_streams(deltas):
    """Pair (dy,dx) with (dy+1,dx); unpaired run as K=64 streams."""
    deltas = sorted(deltas)
    dset, used, streams = set(deltas), set(), []
    for dl in deltas:
        if dl in used:
            continue
        hi = (dl[0] + 1, dl[1])
        if hi in dset and hi not in used:
            streams.append((dl, True))
            used.update((dl, hi))
        else:
            streams.append((dl, False))
            used.add(dl)
    return streams


def _stream_tensors(G, streams):
    """lhsT arrays [K, MROWS] per stream (K=128 paired, 64 single)."""
    out = []
    for dl, paired in streams:
        if paired:
            hi = (dl[0] + 1, dl[1])
            lhsT = np.zeros((128, MROWS), np.float32)
            lhsT[0:C, :] = G[dl].T
            lhsT[C:2 * C, :] = G[hi].T
        else:
            lhsT = np.ascontiguousarray(G[dl].T, dtype=np.float32)
        out.append(lhsT)
    return out


def _plan_and_host_data(d):
    """Everything the host precomputes: stream plans, per-core inputs,
    query routing."""
    cls = _class_constants(d)
    tail_w = np.asarray(d['tail_w'], np.float64)
    tail_b = np.asarray(d['tail_b'], np.float64)

    E_main = _build_E(tail_w, cls)
    deltas = sorted({k for acc in E_main.values() for k in acc})
    dys = [dl[0] for dl in deltas]
    dxs = [dl[1] for dl in deltas]
    dy_min, dy_max = min(dys), max(dys)
    dx_min, dx_max = min(dxs), max(dxs)
    NRF = 17 + dy_max - dy_min       # f rows per core (last row upper-only)
    NCF = W + dx_max - dx_min        # f cols
    NF = NRF * NCF
    assert NRF <= 40 and NCF <= 192, (NRF, NCF)

    main_streams = _pair_streams(deltas)
    G_main = _stack_E(E_main, deltas)
    main_T = _stream_tensors(G_main, main_streams)

    # edge corrections: subtract out-of-image tail-tap contributions
    def corr(only_ty, only_tx, classes, sign):
        E = _build_E(tail_w, cls, only_ty=only_ty, only_tx=only_tx)
        dls = sorted({k for (ba, acc) in E.items() if ba in classes
                      for k, M in acc.items()})
        if not dls:
            return [], []
        st = _pair_streams(dls)
        G = _stack_E(E, dls, classes=classes, sign=sign)
        return st, _stream_tensors(G, st)

    top_cls = [(0, a) for a in range(S)]
    bot_cls = [(3, a) for a in range(S)]
    lef_cls = [(b, 0) for b in range(S)]
    rig_cls = [(b, 3) for b in range(S)]
    c_top = corr((0,), None, top_cls, -1.0)
    c_bot = corr((2,), None, bot_cls, -1.0)
    c_lef = corr(None, (0,), lef_cls, -1.0)
    c_rig = corr(None, (2,), rig_cls, -1.0)
    # corner add-backs: full-row streams; merge reads one column, and the
    # E matrices are zero outside the corner class's 3 rows.
    c_tl = corr((0,), (0,), [(0, 0)], 1.0)
    c_tr = corr((0,), (2,), [(0, 3)], 1.0)
    c_bl = corr((2,), (0,), [(3, 0)], 1.0)
    c_br = corr((2,), (2,), [(3, 3)], 1.0)

    zeros_like_T = lambda Ts: [np.zeros_like(t) for t in Ts]

    # encoder weights: K=28 rows = 9 taps x 3 ch + bias row
    enc_w = np.asarray(d['enc_w'], np.float64)
    enc_b = np.asarray(d['enc_b'], np.float64)
    encw = np.zeros((28, C), np.float32)
    for ty in range(3):
        for tx in range(3):
            for ch in range(3):
                encw[(ty * 3 + tx) * 3 + ch, :] = enc_w[:, ch, ty, tx]
    encw[27, :] = enc_b

    # per-core im2col [28, NRF, NCF]
    inp = np.asarray(d['inp'], np.float64)[0]   # [3, H, W]
    PADX = 64
    ippad = np.pad(inp, ((0, 0), (PADX, PADX), (PADX, PADX)))
    ones = np.zeros((H + 2 * PADX, W + 2 * PADX))
    ones[PADX:PADX + H, PADX:PADX + W] = 1.0
    im2cols = []
    for core in range(NCORES):
        y0 = YLC * core + dy_min          # global LR row of f-tile row 0
        x0 = dx_min
        im = np.zeros((28, NRF, NCF), np.float32)
        for ty in range(3):
            for tx in range(3):
                ys = PADX + y0 + ty - 1
                xs = PADX + x0 + tx - 1
                for ch in range(3):
                    im[(ty * 3 + tx) * 3 + ch] = \
                        ippad[ch, ys:ys + NRF, xs:xs + NCF]
        inside = ones[PADX + y0:PADX + y0 + NRF, PADX + x0:PADX + x0 + NCF]
        im[27] = inside
        # f must be exactly zero at out-of-image positions (grid-sample
        # zero padding), so kill whole columns there, not just oob taps
        im *= inside[None].astype(np.float32)
        im2cols.append(im.reshape(28, NF))

    # query routing (f32 math matches reference rounding)
    coord = np.asarray(d['coord'], np.float32)[0]
    cell = np.asarray(d['cell'], np.float32)[0]
    cq = np.clip(coord - cell * np.float32(0.5) + np.float32(1e-6),
                 np.float32(-1 + 1e-6), np.float32(1 - 1e-6))
    xi = np.clip(np.round((cq[:, 1] + 1) * np.float32(0.5) * (WH - 1)
                          ).astype(np.int64), 0, WH - 1)
    yi = np.clip(np.round((cq[:, 0] + 1) * np.float32(0.5) * (HH - 1)
                          ).astype(np.int64), 0, HH - 1)
    core_of = yi // HRPC
    ylq = (yi % HRPC) // S
    bq = yi % S
    xlq = xi // S
    aq = xi % S
    cls_q = bq * S + aq
    grow = (xlq * YLC + ylq) * 4 + cls_q // 4         # D row, [0, NPIX//4)
    sub_q = cls_q % 4                                 # 3-float slot in row
    NROWS_D = NPIX // 4
    Q = coord.shape[0]
    # Sorted block-gather routing: per core, sort queries by D row, split
    # into 128 equal groups; partition p block-fetches its group's span.
    idx_arrays, originals, subsels, localoff = [], [], [], []
    max_span = 1
    for core in range(NCORES):
        sel = np.nonzero(core_of == core)[0]
        assert sel.size <= NQP, f"core {core} got {sel.size} queries"
        rows = np.full(NQP, 0, np.int64)
        rows[:sel.size] = grow[sel]
        if sel.size:
            rows[sel.size:] = rows[sel.size - 1]
        order = np.argsort(rows[:sel.size], kind='stable')
        rows_sorted = np.concatenate([rows[:sel.size][order],
                                      rows[sel.size:]])
        per = NQP // 128
        lo = rows_sorted.reshape(128, per)[:, 0].copy()
        span = rows_sorted.reshape(128, per)[:, -1] - lo + 1
        max_span = max(max_span, int(span.max()))
        idx_arrays.append(lo.astype(np.int32).reshape(128, 1))
        originals.append(sel[order] if sel.size else sel)
        subsels.append(sub_q[sel][order] if sel.size else sub_q[sel])
        localoff.append((rows_sorted - np.repeat(lo, per)).astype(np.int64))
    BLK = min(NROWS_D, ((max_span + 15) // 16) * 16)
    for core in range(NCORES):
        lo = idx_arrays[core][:, 0].astype(np.int64)
        lo2 = np.minimum(lo, NROWS_D - BLK)
        localoff[core] = localoff[core] + np.repeat(lo - lo2, NQP // 128)
        idx_arrays[core] = lo2.astype(np.int32).reshape(128, 1)
        assert (localoff[core] < BLK).all()

    bias48 = np.zeros((1, MROWS), np.float32)
    for b in range(S):
        for a in range(S):
            bias48[0, (4 * b + a) * 3:(4 * b + a) * 3 + 3] = tail_b

    plan = dict(
        dy_min=dy_min, dx_min=dx_min, NRF=NRF, NCF=NCF, NF=NF,
        main_streams=main_streams,
        corr_specs=dict(top=c_top[0], bot=c_bot[0], lef=c_lef[0],
                        rig=c_rig[0], tl=c_tl[0], tr=c_tr[0],
                        bl=c_bl[0], br=c_br[0]),
    )

    per_core_corr = []
    for core in range(NCORES):
        cc = dict(lef=c_lef[1], rig=c_rig[1])
        cc['top'] = c_top[1] if core == 0 else zeros_like_T(c_top[1])
        cc['tl'] = c_tl[1] if core == 0 else zeros_like_T(c_tl[1])
        cc['tr'] = c_tr[1] if core == 0 else zeros_like_T(c_tr[1])
        cc['bot'] = c_bot[1] if core == NCORES - 1 else zeros_like_T(c_bot[1])
        cc['bl'] = c_bl[1] if core == NCORES - 1 else zeros_like_T(c_bl[1])
        cc['br'] = c_br[1] if core == NCORES - 1 else zeros_like_T(c_br[1])
        per_core_corr.append(cc)

    ones512 = np.ones((1, 512), np.float32)
    plan['BLK'] = BLK
    host = dict(encw=encw, ones512=ones512, im2cols=im2cols, main_T=main_T,
                per_core_corr=per_core_corr, idx_arrays=idx_arrays,
                originals=originals, subsels=subsels, localoff=localoff,
                bias48=bias48, Q=Q)
    return plan, host


def _dma_gather_small_elem(nc, out_ap, in_ap, idxs_ap, num_idxs,
                           elem_size, elem_step, queue_num=0):
    """nc.gpsimd.dma_gather minus the 256-byte *element* restriction.

    The real hardware constraint is that the source ROW STRIDE
    (elem_step * dtype size) is a multiple of 256 bytes; the payload per
    index (elem_size) can be smaller.  Mirrors the non-transpose branch of
    BassGpSimd.dma_gather.
    """
    _ensure_path()
    import concourse.mybir as mybir
    from concourse import ap_utils

    gp = nc.gpsimd
    assert idxs_ap.dtype == mybir.dt.int16
    assert in_ap.dtype == out_ap.dtype
    assert ap_utils.ap_is_contiguous(in_ap.ap[1:])
    assert ap_utils.ap_is_contiguous(out_ap.ap[1:])
    assert ap_utils.ap_is_contiguous(idxs_ap.ap[1:])
    assert in_ap.ap[-1][1] == out_ap.ap[-1][1] == elem_size
    assert out_ap.ap[0][1] * out_ap.ap[1][1] == num_idxs
    assert in_ap.ap[0][0] == elem_step
    stride_bytes = elem_step * mybir.dt.size(in_ap.dtype)
    stride_bytes_256 = stride_bytes // 256
    assert stride_bytes_256 * 256 == stride_bytes and stride_bytes_256 < 256

    _in_ap = gp.lower_ap_dma(in_ap, for_custom_bir_dma=True)
    _idxs_ap = gp.lower_ap(idxs_ap)
    _out_ap = gp.lower_ap(out_ap)
    return gp.add_instruction(
        mybir.InstDMAGatherAnt(
            name=nc.get_next_instruction_name(),
            ins=[*_in_ap, _idxs_ap,
                 gp.lower_val_access(gp.to_reg(num_idxs))],
            outs=[_out_ap],
            transpose=False,
            num_idxs=num_idxs,
            elem_size=elem_size,
            stride_bytes_256=stride_bytes_256,
            gen_mode=0,
            single_packet=True,
            queue_num=queue_num,
            sbuf_tokens_per_rank=0,
            sbuf_free_dim_per_rank=0,
            sbuf_free_dim_pad_per_rank=0,
            sbuf_byte_offset=0,
        ))


def _build_graph(plan, host, debug_outputs=False):
    _ensure_path()
    import concourse.bass as bass
    import concourse.bacc as bacc
    import concourse.mybir as mybir
    import concourse.tile as tile
    from concourse.masks import make_identity

    f32 = mybir.dt.float32
    f32r = mybir.dt.float32r
    i32 = mybir.dt.int32
    i16 = mybir.dt.int16

    NRF, NCF, NF = plan['NRF'], plan['NCF'], plan['NF']
    dy_min, dx_min = plan['dy_min'], plan['dx_min']
    main_streams = plan['main_streams']
    corr_specs = plan['corr_specs']

    nc = bacc.Bacc(None, target_bir_lowering=False, debug=False,
                   num_devices=NCORES)

    im2col_d = nc.dram_tensor('im2col', [28, NF], f32r, kind='ExternalInput')
    encw_d = nc.dram_tensor('encw', [28, C], f32r, kind='ExternalInput')
    bias_d = nc.dram_tensor('bias48', [1, MROWS], f32r, kind='ExternalInput')
    ones_d = nc.dram_tensor('ones512', [1, 512], f32r, kind='ExternalInput')
    idx_d = nc.dram_tensor('idx', [128, 1], i32, kind='ExternalInput')
    mainT_d = [nc.dram_tensor(f'Em{s}', list(t.shape), f32r,
                              kind='ExternalInput')
               for s, t in enumerate(host['main_T'])]
    corrT_d = {}
    for name, streams in corr_specs.items():
        corrT_d[name] = [
            nc.dram_tensor(f'Ec_{name}{s}',
                           [128 if paired else C, MROWS], f32r,
                           kind='ExternalInput')
            for s, (dl, paired) in enumerate(streams)]
    BLK = plan['BLK']
    out_d = nc.dram_tensor('out', [128, BLK * 12], f32,
                           kind='ExternalOutput')
    if debug_outputs:
        dbg_f2 = nc.dram_tensor('dbg_f2', [128, NF], f32, kind='ExternalOutput')
        dbg_pred = nc.dram_tensor('dbg_pred', [MROWS, YLC * W], f32,
                                  kind='ExternalOutput')
        dbg_D = nc.dram_tensor('dbg_D', [128, YLC * MROWS], f32,
                               kind='ExternalOutput')

    with tile.TileContext(nc) as tc:
        with (
            tc.tile_pool(name='sb', bufs=1) as sb,
            tc.tile_pool(name='sbsmall', bufs=1) as sbs,
            tc.tile_pool(name='pshare', bufs=2, space='PSUM') as pshare,
            tc.tile_pool(name='ppred', bufs=1, space='PSUM') as ppred,
            tc.tile_pool(name='pcorr', bufs=1, space='PSUM') as pcorr,
            tc.tile_pool(name='pcorr2', bufs=1, space='PSUM') as pcorr2,
            tc.tile_pool(name='dram', bufs=1, space='DRAM') as dpool,
        ):
            D_t = dpool.tile([NPIX // 4, 12], f32)
            im2col = sb.tile([28, NF], f32r)
            nc.sync.dma_start(im2col[:], im2col_d[:])
            encw_t = sbs.tile([28, C], f32r)
            nc.sync.dma_start(encw_t[:], encw_d[:])
            bias_t = sbs.tile([1, MROWS], f32r)
            nc.sync.dma_start(bias_t[:], bias_d[:])
            idx_t = sb.tile([128, 1], i32)
            nc.sync.dma_start(idx_t[:], idx_d[:])
            mainT_t = []
            for s, td in enumerate(mainT_d):
                t = sbs.tile(list(td.shape), f32r, tag=f'Em{s}')
                nc.sync.dma_start(t[:], td[:])
                mainT_t.append(t)
            corrT_t = {}
            for name, tds in corrT_d.items():
                corrT_t[name] = []
                for s, td in enumerate(tds):
                    t = sbs.tile(list(td.shape), f32r, tag=f'Ec_{name}{s}')
                    nc.sync.dma_start(t[:], td[:])
                    corrT_t[name].append(t)
            ident = sbs.tile([128, 128], f32)
            make_identity(nc, ident[:])
            ones_t = sbs.tile([1, 512], f32r)
            nc.sync.dma_start(ones_t[:], ones_d[:])

            # encoder conv: f2 = [f ; f shifted one LR row]
            f2 = sb.tile([128, NF], f32r)
            CH = 512
            nchunks = (NF + CH - 1) // CH
            for ci in range(nchunks):
                n0, n1 = ci * CH, min(NF, (ci + 1) * CH)
                pe = pshare.tile([C, CH], f32, tag='pshare')
                nc.tensor.matmul(pe[:, :n1 - n0],
                                 encw_t[:],
                                 im2col[:, n0:n1],
                                 start=True, stop=True)
                nc.vector.tensor_copy(f2[0:C, n0:n1], pe[:, :n1 - n0])
                # upper copy, pre-shifted one LR row (NCF elements)
                u0, u1 = n0 - NCF, n1 - NCF
                s0 = max(0, -u0)
                if u1 > 0:
                    nc.scalar.activation(
                        f2[C:128, u0 + s0:u1], pe[:, s0:n1 - n0],
                        mybir.ActivationFunctionType.Copy)

            f3 = f2[:].rearrange('p (r c) -> p r c', c=NCF)

            # main streams into pred psum [48, 2048]
            pred_ps = ppred.tile([MROWS, YLC * W], f32)
            for nb in range(4):
                for s, (dl, paired) in enumerate(main_streams):
                    K = 128 if paired else C
                    r0 = 4 * nb + dl[0] - dy_min
                    c0 = dl[1] - dx_min
                    nc.tensor.matmul(
                        pred_ps[:, nb * 512:(nb + 1) * 512],
                        mainT_t[s][:],
                        f3[0:K, r0:r0 + 4, c0:c0 + W],
                        start=(s == 0), stop=False,
                        skip_group_check=True)
                # tail bias via K=1 matmul of ones
                nc.tensor.matmul(
                    pred_ps[:, nb * 512:(nb + 1) * 512],
                    bias_t[:],
                    ones_t[:],
                    start=False, stop=True, skip_group_check=True)

            # correction psum bank: [top 128 | bot 128 | lef 16 | rig 16 | 4 corners]
            corr_ps = pcorr.tile([MROWS, 512], f32)

            def corr_mms(name, col0, row_sel, col_sel, nfree, ps=None):
                streams = corr_specs[name]
                if ps is None:
                    ps = corr_ps
                if not streams:
                    return False
                for s, (dl, paired) in enumerate(streams):
                    K = 128 if paired else C
                    r0 = row_sel + dl[0] - dy_min
                    c0 = col_sel + dl[1] - dx_min
                    if nfree == 128:     # one row, all cols
                        rhs = f3[0:K, r0:r0 + 1, c0:c0 + W]
                    else:                # all rows, one col
                        rhs = f3[0:K, r0:r0 + YLC, c0:c0 + 1]
                    nc.tensor.matmul(
                        ps[:, col0:col0 + nfree],
                        corrT_t[name][s][:],
                        rhs,
                        start=(s == 0), stop=(s == len(streams) - 1),
                        skip_group_check=True)
                return True

            corr2_ps = pcorr2.tile([MROWS, 512], f32)

            has = dict()
            has['top'] = corr_mms('top', 0, 0, 0, 128)
            has['bot'] = corr_mms('bot', 128, YLC - 1, 0, 128)
            has['lef'] = corr_mms('lef', 256, 0, 0, 16)
            has['rig'] = corr_mms('rig', 272, 0, W - 1, 16)
            has['tl'] = corr_mms('tl', 0, 0, 0, 128, ps=corr2_ps)
            has['tr'] = corr_mms('tr', 128, 0, 0, 128, ps=corr2_ps)
            has['bl'] = corr_mms('bl', 256, YLC - 1, 0, 128, ps=corr2_ps)
            has['br'] = corr_mms('br', 384, YLC - 1, 0, 128, ps=corr2_ps)

            # psum -> sbuf with corrections merged
            pred_sb = sb.tile([MROWS, YLC * W], f32)
            for nb in range(4):
                nc.vector.tensor_copy(pred_sb[:, nb * 512:(nb + 1) * 512],
                                      pred_ps[:, nb * 512:(nb + 1) * 512])
            p4 = pred_sb[:].rearrange('p (r c) -> p r c', c=W)
            if has['top']:
                nc.vector.tensor_add(pred_sb[:, 0:W], pred_sb[:, 0:W],
                                     corr_ps[:, 0:W])
            if has['bot']:
                nc.vector.tensor_add(pred_sb[:, (YLC - 1) * W:YLC * W],
                                     pred_sb[:, (YLC - 1) * W:YLC * W],
                                     corr_ps[:, 128:128 + W])
            if has['lef']:
                nc.vector.tensor_add(
                    p4[:, :, 0:1], p4[:, :, 0:1],
                    corr_ps[:, 256:272].rearrange('p (r c) -> p r c', c=1))
            if has['rig']:
                nc.vector.tensor_add(
                    p4[:, :, W - 1:W], p4[:, :, W - 1:W],
                    corr_ps[:, 272:288].rearrange('p (r c) -> p r c', c=1))
            corner_specs = [
                ('tl', 0, 0, 0 + 0), ('tr', 0, W - 1, 128 + W - 1),
                ('bl', YLC - 1, 0, 256 + 0), ('br', YLC - 1, W - 1,
                                              384 + W - 1)]
            for name, r, cc, scol in corner_specs:
                if not has[name]:
                    continue
                nc.vector.tensor_add(
                    p4[:, r:r + 1, cc:cc + 1],
                    p4[:, r:r + 1, cc:cc + 1],
                    corr2_ps[:, scol:scol + 1]
                    .rearrange('p (r c) -> p r c', c=1))

            # transpose to pixel-major [xl, yl*48 + cls*3 + c]
            D_sb = sb.tile([128, YLC * MROWS], f32)
            for ch in range(YLC):
                pt = pshare.tile([128, MROWS], f32, tag='pshare')
                nc.tensor.transpose(pt[:], pred_sb[:, ch * W:(ch + 1) * W],
                                    ident[0:MROWS, 0:MROWS])
                nc.vector.tensor_copy(D_sb[:, ch * MROWS:(ch + 1) * MROWS],
                                      pt[:])
            nc.sync.dma_start(
                D_t[:].rearrange('(xl r) k -> xl (r k)', xl=128), D_sb[:])
            if debug_outputs:
                nc.sync.dma_start(dbg_f2[:], f2[:].bitcast(f32))
                nc.sync.dma_start(dbg_pred[:], pred_sb[:])
                nc.sync.dma_start(dbg_D[:], D_sb[:])

            # indirect gather of routed queries
            gath = sb.tile([128, BLK * 12], f32)
            nc.gpsimd.indirect_dma_start(
                out=gath[:], out_offset=None,
                in_=D_t[:],
                in_offset=bass.IndirectOffsetOnAxis(ap=idx_t[:], axis=0))
            nc.sync.dma_start(out_d[:], gath[:])

    nc.compile()
    return nc


def make_in_maps(host):
    in_maps = []
    for core in range(NCORES):
        m = {
            'im2col': host['im2cols'][core],
            'encw': host['encw'],
            'bias48': host['bias48'],
            'ones512': host['ones512'],
            'idx': host['idx_arrays'][core],
        }
        for s, t in enumerate(host['main_T']):
            m[f'Em{s}'] = t
        for name, Ts in host['per_core_corr'][core].items():
            for s, t in enumerate(Ts):
                m[f'Ec_{name}{s}'] = t
        in_maps.append(m)
    return in_maps


def kernel(**inputs) -> np.ndarray:
    _ensure_path()
    from concourse.bass_utils import run_bass_kernel_spmd

    scale = inputs.get('scale', S)
    scale = int(np.asarray(scale)) if not isinstance(scale, int) else scale
    assert scale == S, f"kernel hardcodes scale={S}, got {scale}"

    plan, host = _plan_and_host_data(inputs)
    nc = _build_graph(plan, host)

    in_maps = make_in_maps(host)
    res = run_bass_kernel_spmd(nc, in_maps, core_ids=list(range(NCORES)))

    Q = host['Q']
    BLK = plan['BLK']
    q = np.zeros((Q, 3), np.float32)
    per = NQP // 128
    for core in range(NCORES):
        sel = host['originals'][core]        # original ids, sorted order
        sub = host['subsels'][core]
        loc = host['localoff'][core]
        blocks = np.asarray(res.results[core]['out']).reshape(128, BLK * 12)
        n = sel.size
        prt = (np.arange(n) // per)
        base = loc[:n] * 12 + sub * 3
        cols = base[:, None] + np.arange(3)[None]
        q[sel] = np.take_along_axis(blocks[prt], cols, axis=1)
    return q[None]
